# revision 1
# baseline (speedup 1.0000x reference)
"""Linear-chain CRF forward pass on 8 Trainium2 NeuronCores.

Reference recurrence (per batch element b):
    alpha_t[j] = x_t[j] + logsumexp_k(alpha_{t-1}[k] + trans[j,k])
    out[b] = sum_j alpha_{L_b - 1}[j]

Device formulation: exp space with a constant per-step log shift c folded
into the transition matrix:
    E_t = (Mc @ E_{t-1}) * X_t,  Mc[j,k] = exp(trans[j,k] - c),  X_t = exp(x_t)
so alpha_t = log E_r + r*c + A for a per-trajectory constant A.

The 2048-step serial chain is broken via the Birkhoff contraction of the
positive map E -> Mc @ E (contraction <= tanh(spread(trans)/2) ~ 0.46 per
step; elementwise positive scalings are Hilbert-metric isometries): time is
cut into 32 segments of 64 steps, each warmed up W rounds from an arbitrary
positive init.  The unknown per-segment offsets A_s are recovered on the
host by telescoping mean log-ratios at segment boundaries, where the value
is computed by both the owning segment and its predecessor.

Per-core layout (32 batch elements/core, data-parallel over batch):
  State E[row, col]: 128 partitions = 2 row-blocks x 64 classes, 256
  columns = 8 segment-blocks x 32 local b.  Two independent instruction
  chains ("pairs"), each advanced per round by one K=128 block-diagonal
  float32r matmul (N=256) plus one (128,256) DVE multiply.
  Segment s = 16*pair + 8*rowblock + block.
  Segment 0 replays the exact trajectory from t=0 (true init
  exp(x_0 + origination) injected via its round-0 X columns).
"""

from contextlib import ExitStack

import numpy as np

B, T, C = 256, 2048, 64
NCORES = 8
BPC = B // NCORES          # 32
SEG = 32
SEG_LEN = T // SEG         # 64
W = 10                     # warmup rounds for segments s >= 1
L = SEG_LEN + W + 1        # 75 rounds; round 0 = init
PAIRS = 2
NCOL = 256
CHUNK = 5                  # rounds per exp chunk; CHUNK divides L
DCH = 5                  # rounds per DMA chunk; CHUNK divides DCH divides L
# Stitch rounds: segments s-1 and s share global t = 64s - 2 at local
# rounds 72 and 8 (segment 0: t = 62 at round 62).  Kept off the final
# round so the snapshot drain overlaps the last rounds of compute.
STITCH_J = W - 2
SNAP_ROUNDS = (STITCH_J, SEG_LEN - 2, SEG_LEN + W - 2)

_CACHE = {}


def _c_step(transitions, pad_x):
    """Mean per-step growth of max_j alpha, from a short host simulation."""
    x = np.asarray(pad_x[:4], np.float64)
    tr = np.asarray(transitions, np.float64)
    a = x[:, 0, :]
    tot, n = 0.0, 0
    for t in range(1, 257):
        s = a[:, None, :] + tr[None, :, :]
        m = s.max(axis=2, keepdims=True)
        a_new = x[:, t, :] + np.log(np.exp(s - m).sum(axis=2)) + m[:, :, 0]
        tot += float((a_new.max(axis=1) - a.max(axis=1)).mean())
        n += 1
        a = a_new
    return tot / n


def _seg_of(t_star):
    return min(t_star // SEG_LEN, SEG - 1)


def _round_of(t_star):
    s = _seg_of(t_star)
    return t_star if s == 0 else t_star - s * SEG_LEN + W


def _col_of(s, b=0):
    p, rem = divmod(s, 16)
    h, q = divmod(rem, 8)
    return p, h, q * 32 + b


def _build_host_inputs(pad_x, transitions, origination, c):
    """X_raw per core: [PAIRS, 128, L*NCOL] f32 laid out so each partition
    row is contiguous over (round, col); exp is applied on device.  Also the
    block-diagonal lhsT weights [128, 128] f32."""
    mc = np.exp(np.asarray(transitions, np.float64) - c).astype(np.float32)
    wmat = np.zeros((128, 128), np.float32)
    wmat[:64, :64] = mc.T      # lhsT[k, j] = Mc[j, k]
    wmat[64:, 64:] = mc.T

    x0 = np.asarray(pad_x, np.float32).copy()
    x0[:, 0, :] += np.asarray(origination, np.float32)[None, :]
    xc = x0.reshape(NCORES, BPC, T, C)

    import ml_dtypes
    xraw = np.zeros((NCORES, PAIRS, 128, L, NCOL), ml_dtypes.bfloat16)
    for s in range(SEG):
        t0 = 0 if s == 0 else s * SEG_LEN - W
        t_idx = np.arange(L) + t0
        valid = (t_idx >= 0) & (t_idx < T)
        t_clip = np.clip(t_idx, 0, T - 1)
        p, h, col0 = _col_of(s)
        # (core, b, L, C) -> (core, C, L, b)
        blk = xc[:, :, t_clip, :] * valid[None, None, :, None]
        xraw[:, p, 64 * h:64 * h + 64, :, col0:col0 + 32] = \
            blk.transpose(0, 3, 2, 1).astype(ml_dtypes.bfloat16)
    return xraw.reshape(NCORES, PAIRS, 128, L * NCOL), wmat


def _extraction_schedule(batch_sizes):
    """Per-core static extraction events (round, pair, rowblock, col,
    global_b).  The SPMD program is shared, so the device executes the
    union of all cores' events (each into its own fin column, keyed by
    global b); each core's host-side readback uses only its own events."""
    bs = np.asarray(batch_sizes).reshape(NCORES, BPC)
    sched = []
    for core in range(NCORES):
        ev = []
        for b in range(BPC):
            t_star = int(bs[core, b]) - 1
            s = _seg_of(t_star)
            r = _round_of(t_star)
            p, h, col = _col_of(s, b)
            ev.append((r, p, h, col, core * BPC + b))
        sched.append(ev)
    return sched


def _build_program(by_round):
    """Raw-bass program with explicit per-engine streams and standalone
    semaphore waits (DVE instructions only support ONE embedded sync wait on
    this toolchain, so Tile's embedded-wait scheduling cannot compile the
    tight mm->mul loop).  by_round: round -> [(p, h, col, global_b)].

    Engine streams:
      SP   : weight DMA, X chunk DMAs, snapshot DMAs, final fin DMA
      ACT  : f32r rounding copy of weights, exp of X chunks
      PE   : 2 block-diagonal f32r matmuls per round
      DVE  : 2 (128, NCOL) multiplies per round (the bottleneck engine --
             nothing else runs here)
      Pool : fin column extractions and snapshot staging copies
    """
    import concourse.bass as bass
    from concourse import mybir

    dt = mybir.dt
    NCH = L // CHUNK          # exp chunks
    NDC = L // DCH            # DMA chunks
    SPD = DCH // CHUNK        # exp chunks per DMA chunk
    ERING = 8
    nc = bass.Bass()
    xp = nc.declare_dram_parameter("xp", [PAIRS, 128, L * NCOL], dt.bfloat16,
                                   False)
    wm = nc.declare_dram_parameter("wm", [128, 128], dt.float32, False)
    snaps = nc.declare_dram_parameter(
        "snaps", [len(SNAP_ROUNDS) * PAIRS, 128, NCOL], dt.float32r, True)
    fin = nc.declare_dram_parameter("fin", [64, B], dt.float32r, True)

    with ExitStack() as ctx:
        def sb(name, shape, d):
            return ctx.enter_context(nc.sbuf_tensor(name, shape, d))
        wraw = sb("wraw", [128, 128], dt.float32)
        wr = sb("wr", [128, 128], dt.float32r)
        raw = [[sb(f"raw{i}_{p}", [128, DCH * NCOL], dt.bfloat16)
                for p in range(PAIRS)] for i in range(2)]
        xr = [[sb(f"xr{i}_{p}", [128, CHUNK * NCOL], dt.bfloat16)
               for p in range(PAIRS)] for i in range(3)]
        e0 = [sb(f"e0_{p}", [128, NCOL], dt.float32r) for p in range(PAIRS)]
        et = [[sb(f"et{p}_{i}", [128, NCOL], dt.float32r)
               for i in range(ERING)] for p in range(PAIRS)]
        # write-once staging for segment-boundary snapshots; DVE fills them
        # right after the snapshot round, SP drains them at the end
        snapst = [sb(f"snapst{i}", [128, NCOL], dt.float32r)
                  for i in range(len(SNAP_ROUNDS) * PAIRS)]
        fin_t = sb("fin_t", [64, B], dt.float32r)
        ps = [[ctx.enter_context(
            nc.psum_tensor(f"ps{p}_{i}", [128, NCOL], dt.float32))
            for i in range(2)] for p in range(PAIRS)]
        psd = ctx.enter_context(
            nc.psum_tensor("psd", [128, NCOL], dt.float32))
        s_w = ctx.enter_context(nc.semaphore("s_w"))
        s_x0 = ctx.enter_context(nc.semaphore("s_x0"))
        s_x1 = ctx.enter_context(nc.semaphore("s_x1"))
        s_x2 = ctx.enter_context(nc.semaphore("s_x2"))
        s_x = (s_x0, s_x1, s_x2)
        NSX = len(s_x)
        s_a = ctx.enter_context(nc.semaphore("s_a"))
        s_pe = ctx.enter_context(nc.semaphore("s_pe"))
        s_v = ctx.enter_context(nc.semaphore("s_v"))
        s_f = ctx.enter_context(nc.semaphore("s_f"))
        s_s = ctx.enter_context(nc.semaphore("s_s"))
        s_o = ctx.enter_context(nc.semaphore("s_o"))
        s_so0 = ctx.enter_context(nc.semaphore("s_so0"))
        s_so1 = ctx.enter_context(nc.semaphore("s_so1"))
        s_so = (s_so0, s_so1)
        block = ctx.enter_context(nc.Block())

        def xsl(p, r):
            k, rr = divmod(r, CHUNK)
            return xr[k % 3][p][:, rr * NCOL:(rr + 1) * NCOL]

        def ecur(p, r):
            return et[p][r % ERING]

        def eprev(p, r):
            if r == 1:
                return e0[p][:]
            return ecur(p, r - 1)[:]

        def act_cnt(k, p):
            # s_a value after exp(k, p): chunk-0 exps first, then the weight
            # rounding copy, two f32r E0-init exps, then chunk k>=1 exps
            return p + 1 if k == 0 else 2 * k + p + 4

        nfin = sum(len(v) for v in by_round.values())
        # cumulative Pool-copy counts per round, for cross-engine WAR waits
        cumfin = [0] * L
        cumsnap = [0] * L
        for r in range(L):
            prev_f = cumfin[r - 1] if r else 0
            prev_s = cumsnap[r - 1] if r else 0
            cumfin[r] = prev_f + len(by_round.get(r, ()))
            cumsnap[r] = prev_s + (PAIRS if r in SNAP_ROUNDS else 0)

        @block.sync
        def _(sync):
            sync.dma_start(wraw[:], wm[:, :]).then_inc(s_w, 16)
            for k in range(NDC):
                for p in range(PAIRS):
                    n = 2 * k + p
                    if k >= 2:
                        # raw slot reused; all exps of DMA chunk k-2 done
                        # (for k==2 the E0-init exps also read raw[0])
                        sync.wait_ge(s_a, max(
                            act_cnt(SPD * (k - 2) + SPD - 1, 1),
                            5 if k == 2 else 0))
                    if n >= NSX:
                        # DMA-completion ordering protocol for the shared sem
                        sync.wait_ge(s_x[n % NSX], 16 * (n // NSX))
                    sync.dma_start(
                        raw[k % 2][p][:],
                        xp[p, :, k * DCH * NCOL:(k + 1) * DCH * NCOL],
                    ).then_inc(s_x[n % NSX], 16)
            sync.wait_ge(s_f, nfin)
            sync.dma_start(fin[:, :], fin_t[:]).then_inc(s_o, 16)

        @block.scalar
        def _(scalar):
            for k in range(NCH):
                kd, ks = divmod(k, SPD)   # DMA chunk, sub-chunk within it
                for p in range(PAIRS):
                    n = 2 * kd + p
                    if ks == 0:
                        scalar.wait_ge(s_x[n % NSX], 16 * (n // NSX + 1))
                    if k >= 3:
                        # xr slot reused; all muls of chunk k-3 done
                        scalar.wait_ge(s_v, 2 * (CHUNK * (k - 3) + CHUNK - 1))
                    nc.scalar.activation(
                        xr[k % 3][p][:],
                        raw[kd % 2][p][:, ks * CHUNK * NCOL:
                                        (ks + 1) * CHUNK * NCOL],
                        mybir.ActivationFunctionType.Exp).then_inc(s_a, 1)
                if k == 0:
                    scalar.wait_ge(s_w, 16)
                    nc.scalar.copy(wr[:], wraw[:]).then_inc(s_a, 1)
                    # f32r E0 init (bf16 xr cannot feed the f32r matmul)
                    for p in range(PAIRS):
                        nc.scalar.activation(
                            e0[p][:], raw[0][p][:, 0:NCOL],
                            mybir.ActivationFunctionType.Exp).then_inc(s_a, 1)

        @block.tensor
        def _(tensor):
            for r in range(1, L):
                for p in range(PAIRS):
                    if r == 1:
                        tensor.wait_ge(s_a, 4 + p)
                    else:
                        tensor.wait_ge(s_v, 2 * (r - 2) + p + 1)
                    nc.tensor.matmul(ps[p][r % 2][:], wr[:], eprev(p, r),
                                     start=True, stop=True).then_inc(s_pe, 1)
                if r >= 2:
                    # keep the PE p-state ramped: filler matmuls on static
                    # inputs fill the idle window between dependent rounds
                    for _ in range(2):
                        nc.tensor.matmul(psd[:], wr[:], e0[0][:],
                                         start=True, stop=True)

        @block.vector
        def _(vector):
            for r in range(1, L):
                if r == 1:
                    vector.wait_ge(s_a, 2)
                elif r % CHUNK == 0:
                    vector.wait_ge(s_a, act_cnt(r // CHUNK, 1))
                if r >= ERING:
                    # Pool copies reading the ring slot this round reuses
                    w_r = r - ERING
                    if cumfin[w_r] > (cumfin[w_r - 1] if w_r else 0):
                        vector.wait_ge(s_f, cumfin[w_r])
                for p in range(PAIRS):
                    vector.wait_ge(s_pe, 2 * (r - 1) + p + 1)
                    nc.vector.tensor_mul(ecur(p, r)[:],
                                         ps[p][r % 2][:],
                                         xsl(p, r)).then_inc(s_v, 1)
                if r in SNAP_ROUNDS:
                    si = SNAP_ROUNDS.index(r)
                    # DVE is pipelined: wait for this round's muls to retire
                    vector.wait_ge(s_v, 2 * r)
                    for p in range(PAIRS):
                        nc.vector.tensor_copy(
                            snapst[2 * si + p][:],
                            ecur(p, r)[:]).then_inc(s_s, 1)

        @block.gpsimd
        def _(gpsimd):
            for r in range(L):
                for (p, h, col, gb) in by_round.get(r, ()):
                    if r == 0:
                        gpsimd.wait_ge(s_a, 4 + p)
                        src = e0[p][64 * h:64 * h + 64, col:col + 1]
                    else:
                        gpsimd.wait_ge(s_v, 2 * (r - 1) + p + 1)
                        src = ecur(p, r)[64 * h:64 * h + 64, col:col + 1]
                    nc.gpsimd.tensor_copy(fin_t[:, gb:gb + 1],
                                          src).then_inc(s_f, 1)
                if r in SNAP_ROUNDS:
                    si = SNAP_ROUNDS.index(r)
                    for p in range(PAIRS):
                        i = 2 * si + p
                        gpsimd.wait_ge(s_s, i + 1)
                        if i >= 2:
                            # completion-order protocol for the shared sem
                            gpsimd.wait_ge(s_so[i % 2], 16 * (i // 2))
                        nc.gpsimd.dma_start(
                            snaps[i], snapst[i][:]).then_inc(s_so[i % 2], 16)

    return nc


def _postprocess(snaps, fin, sched_core, c):
    """Per-core host math (float64): stitch segment offsets, read finals."""
    ls = np.log(np.maximum(np.asarray(snaps, np.float64), 1e-300))
    snap = {r: ls[2 * i:2 * i + 2] for i, r in enumerate(SNAP_ROUNDS)}

    def seg_cols(arr, s):
        p, h, col0 = _col_of(s)
        return arr[p][64 * h:64 * h + 64, col0:col0 + 32]  # (64, 32)

    A = np.zeros((SEG, BPC))
    for s in range(1, SEG):
        if s == 1:
            prev, i_prev = seg_cols(snap[SNAP_ROUNDS[1]], 0), SNAP_ROUNDS[1]
        else:
            prev, i_prev = seg_cols(snap[SNAP_ROUNDS[2]], s - 1), \
                SNAP_ROUNDS[2]
        cur = seg_cols(snap[SNAP_ROUNDS[0]], s)
        d = (prev + i_prev * c) - (cur + SNAP_ROUNDS[0] * c)
        A[s] = A[s - 1] + d.mean(axis=0)

    lf = np.log(np.maximum(np.asarray(fin, np.float64), 1e-300))  # (64, B)
    res = np.empty(BPC)
    for (r, p, h, col, gb) in sched_core:
        s = 16 * p + 8 * h + col // 32
        b = gb % BPC
        res[b] = lf[:, gb].sum() + 64.0 * (r * c + A[s, b])
    return res


def kernel(pad_x, transitions, origination, batch_sizes):
    from concourse.bass_utils import run_bass_kernel_spmd

    pad_x = np.asarray(pad_x)
    transitions = np.asarray(transitions)
    origination = np.asarray(origination)
    batch_sizes = np.asarray(batch_sizes)

    c = _c_step(transitions, pad_x)
    xraw, wmat = _build_host_inputs(pad_x, transitions, origination, c)
    sched = _extraction_schedule(batch_sizes)

    by_round = {}
    for ev in sched:
        for (r, p, h, col, gb) in ev:
            by_round.setdefault(r, []).append((p, h, col, gb))

    key = (batch_sizes.tobytes(), round(float(c), 9))
    if key not in _CACHE:
        _CACHE[key] = _build_program(by_round)
    nc = _CACHE[key]

    in_maps = [{"xp": xraw[i], "wm": wmat} for i in range(NCORES)]
    out = run_bass_kernel_spmd(nc, in_maps, list(range(NCORES)))

    res = np.empty(B, np.float32)
    for i in range(NCORES):
        r = _postprocess(out.results[i]["snaps"], out.results[i]["fin"],
                         sched[i], c)
        res[i * BPC:(i + 1) * BPC] = r.astype(np.float32)
    return res



# revision 21
# speedup vs baseline: 2.6784x; 2.6784x over previous
"""Linear-chain CRF forward pass on 8 Trainium2 NeuronCores.

Reference recurrence (per batch element b):
    alpha_t[j] = x_t[j] + logsumexp_k(alpha_{t-1}[k] + trans[j,k])
    out[b] = sum_j alpha_{L_b - 1}[j]

Device formulation: exp space with a constant per-step log shift c folded
into the transition matrix:
    E_r = (Mc @ E_{r-1}) * X_r,  Mc[j,k] = exp(trans[j,k] - c)
so alpha_{t(r)} = log E_r + r*c + A for a per-cell constant A.

The 2048-step serial chain is broken via the Birkhoff contraction of the
positive map E -> Mc @ E: time is cut into segments of SL=16 steps; each
(segment, batch) pair is a "cell" (64 classes) warmed up W rounds from an
all-ones init.  Only VALID cells (t < L_b + warmup margin) are packed, so
length padding costs neither compute nor DMA.  The unknown per-cell
offsets A are recovered on the host by telescoping mean log-ratios at
shared timesteps (two per boundary, averaged), read from snapshot DMAs of
the write-once E history at rounds {RC} (segment side) / {RP}
(predecessor side).

Per-core layout: cells pack 2 per column (row blocks 0:64 / 64:128).
Columns are split across 4 independent chains: elementwise muls for 2
chains on DVE and 2 on Pool (GPSIMD), all matmuls on PE (block-diagonal
bf16 lhsT), extraction staging copies on ACT.  Every cross-engine dep is
an embedded single wait (wait_op); X arrival uses per-chunk standalone
waits.  Segment 0 replays the exact trajectory: its round-(W+1) X column
is exp(alpha_0) / (Mc^{W+1} @ 1) so E[W+1] == exp(alpha_0) exactly.
Cells whose extraction would land on the last two rounds get a phantom
successor cell extracted during its warmup instead, keeping extraction
<= round W+PH_M so the final staging DMA overlaps the tail rounds.
"""

from contextlib import ExitStack

import numpy as np
import ml_dtypes

B, T, C = 256, 2048, 64
NRT = 1                    # routed (ACT-copy + DVE-2x) chains
NCORES = 8
BPC = B // NCORES          # 32
XCOL = BPC // 2            # extraction columns in chain 0 (16)

_CACHE = {}


def set_config(w=6, sl=16, rc0=3, ph_m=13):
    """(Re)derive the round-structure constants.  Rounds r = 2..L-1 execute
    on device (round 1 is folded into X on the host: E[1] == X'[1])."""
    g = globals()
    g["W"], g["SL"] = w, sl
    g["L"] = sl + w + 1
    g["NR"] = g["L"] - 1
    g["RC"] = (rc0, rc0 + 1)          # stitch rounds, current segment
    g["RP"] = (rc0 + sl, rc0 + sl + 1)  # stitch rounds, predecessor
    g["PH_M"] = ph_m                  # m >= PH_M -> phantom successor cell
    g["R_EX0"] = ph_m - sl + w + 1    # earliest extraction round
    g["R_EX1"] = ph_m + w             # latest extraction round (m+W+1)
    g["N_EX"] = g["R_EX1"] - g["R_EX0"] + 1
    # rounds beyond every consumer (stitch service, extraction) are dead
    g["RL"] = max(g["RP"][1], g["R_EX1"])
    assert g["R_EX0"] >= 2 and g["RL"] <= g["L"] - 1, (w, sl, rc0, ph_m)


set_config(w=7, rc0=1, ph_m=11)


def _c_step(transitions, pad_x):
    """Mean per-step growth of max_j alpha, from a short host simulation."""
    x = np.asarray(pad_x[:4], np.float64)
    tr = np.asarray(transitions, np.float64)
    a = x[:, 0, :]
    tot, n = 0.0, 0
    for t in range(1, 257):
        s = a[:, None, :] + tr[None, :, :]
        m = s.max(axis=2, keepdims=True)
        a_new = x[:, t, :] + np.log(np.exp(s - m).sum(axis=2)) + m[:, :, 0]
        tot += float((a_new.max(axis=1) - a.max(axis=1)).mean())
        n += 1
        a = a_new
    return tot / n


def _plan(batch_sizes):
    """Cell enumeration + column assignment, shared across cores."""
    bs = np.asarray(batch_sizes, np.int64)
    cores = [[] for _ in range(NCORES)]          # all cells per core
    extr = [[] for _ in range(NCORES)]           # (s_ex, b, r_ex) per core
    for b in range(B):
        core = b % NCORES
        t_star = int(bs[b]) - 1
        s_max, m = divmod(t_star, SL)
        phantom = m >= PH_M
        for s in range(s_max + 1 + (1 if phantom else 0)):
            cores[core].append((s, b))
        if phantom:
            extr[core].append((s_max + 1, b, m - SL + W + 1))
        else:
            extr[core].append((s_max, b, m + W + 1))
    ncells = max(len(c) for c in cores)
    tc = XCOL + (max(0, ncells - 2 * XCOL) + 1) // 2   # total columns
    tc = max(tc, XCOL + 4)
    # 2 direct DVE chains + NRT routed chains (ACT copies PSUM->SBUF bf16,
    # then DVE multiplies in 2x mode).  GPSIMD cannot read PSUM and ACT
    # cannot run TensorTensor, so ACT's idle copy bandwidth is the only way
    # to feed a second mul path.  Calibrated round model:
    #   DVE busy = 2*(125+1.0417*Nd) + NRT*(60+0.52*Nr)
    #   ACT busy = NRT*(185+0.833*Nr);  routed latency = 792+1.353*Nr
    best = None
    for nr in range(0, tc // 2 + 1):
        nd = (tc - NRT * nr + 1) // 2
        if nd < XCOL or (NRT * nr and nr < 8):
            continue
        dve = 2 * (125 + 1.0417 * nd) + NRT * (60 + 0.52 * nr)
        act = NRT * (185 + 0.833 * nr)
        r = max(dve, act, 518 + 1.0417 * nd, (792 + 1.353 * nr) if nr else 0)
        if best is None or r < best[0]:
            best = (r, nd, nr)
    _, nd, nr = best
    chains = [nd, tc - NRT * nr - nd] + [nr] * (NRT if nr else 0)
    n_chain = len(chains)

    placements = []
    for core in range(NCORES):
        place = {}
        ex = sorted(extr[core], key=lambda e: e[1])   # by global b
        assert len(ex) == BPC
        ex_set = set()
        for slot, (s, b, r_ex) in enumerate(ex):
            place[(s, b)] = (0, slot // 2, slot % 2)
            ex_set.add((s, b))
        rest = [cell for cell in cores[core] if cell not in ex_set]
        free = [(0, j) for j in range(XCOL, chains[0])]
        for cc in range(1, n_chain):
            free += [(cc, j) for j in range(chains[cc])]
        slots = [(cc, jj, hh) for (cc, jj) in free for hh in (0, 1)]
        assert len(rest) <= len(slots), (len(rest), len(slots))
        for cell, sl_ in zip(rest, slots):
            place[cell] = sl_
        placements.append(place)
    return dict(chains=chains, placements=placements, extr=extr)


def _build_host_inputs(pad_x, transitions, origination, c, plan):
    """xp per core: [128, 128 + NR*NC] bf16 = [wmat | X rounds 1..NR]."""
    chains = plan["chains"]
    NC = sum(chains)
    off = np.cumsum([0] + chains)

    mc = np.exp(np.asarray(transitions, np.float64) - c)
    wmat = np.zeros((128, 128), np.float64)
    wmat[:64, :64] = mc.T
    wmat[64:, 64:] = mc.T

    u1 = mc @ np.ones(64, np.float64)      # baked into X'[1] == E[1]
    u = np.ones(64, np.float64)            # Mc^{W+1} @ 1
    for _ in range(W + 1):
        u = mc @ u

    x = np.asarray(pad_x, np.float64)
    alpha0 = x[:, 0, :] + np.asarray(origination, np.float64)[None, :]

    xp = np.ones((NCORES, 128, 128 + NR * NC), np.float64)
    xp[:, :, :128] = wmat[None]
    for core in range(NCORES):
        for (s, b), (cc, jj, hh) in plan["placements"][core].items():
            col = 128 + np.arange(NR) * NC + off[cc] + jj
            rows = slice(64 * hh, 64 * hh + 64)
            t0 = SL * s - W - 1
            ts = t0 + np.arange(1, L)
            xs = np.ones((NR, 64))
            vt = (ts >= (1 if s == 0 else 0)) & (ts < T)
            xs[vt] = np.exp(x[b, ts[vt]])
            if s == 0:
                xs[W] = np.exp(alpha0[b]) / u
            xs[0] = xs[0] * u1
            # mixed advanced/basic indexing puts the col axis first
            xp[core, rows, col] = xs
    return xp.astype(ml_dtypes.bfloat16), NC


def _build_program(plan):
    import concourse.bass as bass
    from concourse import mybir

    dt = mybir.dt
    chains = plan["chains"]
    n_chain = len(chains)
    n_rt = n_chain - 2                   # routed chains (ACT copy + DVE 2x)
    NC = sum(chains)
    off = np.cumsum([0] + chains)
    CHUNKS = [(1, 1), (2, 2), (3, 3), (4, 5), (6, 8), (9, 12), (13, 16),
              (17, RL)]
    N_EARLY = 3                     # chunks issued in the entry BB
    NSX = 3

    nc = bass.Bass()
    xp = nc.declare_dram_parameter("xp", [128, 128 + NR * NC],
                                   dt.bfloat16, False)
    snaps = nc.declare_dram_parameter("snaps", [128, 4 * NC], dt.bfloat16,
                                      True)
    stage_o = nc.declare_dram_parameter("stage", [128, N_EX * XCOL],
                                        dt.bfloat16, True)

    with ExitStack() as ctx:
        def sb(name, shape, d):
            return ctx.enter_context(nc.sbuf_tensor(name, shape, d))
        xbuf = sb("xbuf", [128, 128 + NR * NC], dt.bfloat16)
        wr = xbuf[:, 0:128]
        Eall = sb("eall", [128, (RL + 1) * NC], dt.bfloat16)
        stage = sb("stage_t", [128, N_EX * XCOL], dt.bfloat16)
        # write-once SBUF copies of routed chains' matmul results
        psb = [sb(f"psb{j}", [128, (RL + 1) * chains[2 + j]], dt.bfloat16)
               for j in range(n_rt)]
        ps = [ctx.enter_context(nc.psum_tensor(f"ps{cc}", [128, n],
                                               dt.float32))
              for cc, n in enumerate(chains)]
        s_x = [ctx.enter_context(nc.semaphore(f"s_x{i}")) for i in range(NSX)]
        s_pe = ctx.enter_context(nc.semaphore("s_pe"))
        s_vd = ctx.enter_context(nc.semaphore("s_vd"))
        s_ac = ctx.enter_context(nc.semaphore("s_ac"))
        s_st = ctx.enter_context(nc.semaphore("s_st"))
        s_o = ctx.enter_context(nc.semaphore("s_o"))

        def esl(cc, r):
            base = r * NC + off[cc]
            return Eall[:, base:base + chains[cc]]

        def psl(j, r):
            n = chains[2 + j]
            return psb[j][:, r * n:(r + 1) * n]

        def xsl(cc, r):
            base = 128 + (r - 1) * NC + off[cc]
            return xbuf[:, base:base + chains[cc]]

        def chunk_of(r):
            for k, (r0, r1) in enumerate(CHUNKS):
                if r0 <= r <= r1:
                    return k
            raise AssertionError(r)

        def xtarget(k):
            return 16 * (k // NSX + 1)

        def issue_chunk(eng, k):
            r0, r1 = CHUNKS[k]
            a = 0 if k == 0 else 128 + (r0 - 1) * NC
            bnd = 128 + r1 * NC
            d = eng.dma_start(xbuf[:, a:bnd], xp[:, a:bnd])
            if k >= NSX:
                d.wait_op(s_x[k % NSX], 16 * (k // NSX), "sem-ge")
            d.then_inc(s_x[k % NSX], 16)

        for k in range(N_EARLY):
            issue_chunk(nc.sync, k)

        block = ctx.enter_context(nc.Block())

        @block.sync
        def _(sync):
            for k in range(N_EARLY, len(CHUNKS)):
                issue_chunk(sync, k)
            # stitch snapshots (predecessor side) + extraction staging out
            for k in (0, 1):
                sync.wait_ge(s_vd, n_chain * (RP[k] - 1))
                sync.dma_start(snaps[:, (2 + k) * NC:(3 + k) * NC],
                               Eall[:, RP[k] * NC:(RP[k] + 1) * NC]
                               ).then_inc(s_o, 16)
            sync.dma_start(stage_o[:, :], stage[:]).wait_op(
                s_st, N_EX, "sem-ge").then_inc(s_o, 16)

        @block.tensor
        def _(tensor):
            # p-state warmup: PE ramps to full clock after 3us of activity
            for _ in range(7):
                nc.tensor.matmul(ps[0][:], Eall[:, 0:128].bitcast(
                    mybir.dt.bfloat16), Eall[:, 0:chains[0]],
                    start=True, stop=True)
            for r in range(2, RL + 1):
                for cc in range(n_chain):
                    rhs = xsl(cc, 1) if r == 2 else esl(cc, r - 1)
                    mm = nc.tensor.matmul(ps[cc][:], wr, rhs,
                                          start=True, stop=True)
                    if r == 2:
                        mm.wait_op(s_x[0], 16, "sem-ge")
                    else:
                        # mul(r-1) done implies ps[cc] free and E[r-1] ready
                        mm.wait_op(s_vd, n_chain * (r - 3) + cc + 1, "sem-ge")
                    mm.then_inc(s_pe, 1)

        @block.scalar
        def _(scalar):
            # PSUM -> SBUF bf16 rematerialization feeding the DVE 2x muls
            for r in range(2, RL + 1):
                for j in range(n_rt):
                    cp = nc.scalar.copy(psl(j, r), ps[2 + j][:])
                    cp.wait_op(s_pe, n_chain * (r - 2) + 2 + j + 1, "sem-ge")
                    cp.then_inc(s_ac, 1)

        @block.vector
        def _(vector):
            last_k = -1
            for r in range(2, RL + 1):
                k = chunk_of(r)
                if k != last_k:
                    vector.wait_ge(s_x[k % NSX], xtarget(k))
                    last_k = k
                for cc in range(n_chain):
                    if cc < 2:
                        t = nc.vector.tensor_mul(esl(cc, r), ps[cc][:],
                                                 xsl(cc, r))
                        t.wait_op(s_pe, n_chain * (r - 2) + cc + 1, "sem-ge")
                    else:
                        t = nc.vector.tensor_mul(esl(cc, r), psl(cc - 2, r),
                                                 xsl(cc, r))
                        t.wait_op(s_ac, n_rt * (r - 2) + (cc - 2) + 1,
                                  "sem-ge")
                    t.then_inc(s_vd, 1)

        @block.gpsimd
        def _(gpsimd):
            # stitch snapshot (segment side) + extraction staging copies
            if RC[0] == 1:
                # round 1 is E[1] == X'[1], resident in xbuf, not Eall
                gpsimd.wait_ge(s_x[0], 16)
                gpsimd.dma_start(snaps[:, 0:NC], xbuf[:, 128:128 + NC]
                                 ).then_inc(s_o, 16)
                gpsimd.wait_ge(s_vd, n_chain * (RC[1] - 1))
                gpsimd.dma_start(snaps[:, NC:2 * NC],
                                 Eall[:, RC[1] * NC:(RC[1] + 1) * NC]
                                 ).then_inc(s_o, 16)
            else:
                gpsimd.wait_ge(s_vd, n_chain * (RC[1] - 1))
                gpsimd.dma_start(snaps[:, 0:2 * NC],
                                 Eall[:, RC[0] * NC:(RC[1] + 1) * NC]
                                 ).then_inc(s_o, 16)
            for i, r in enumerate(range(R_EX0, R_EX1 + 1)):
                cp = nc.gpsimd.tensor_copy(stage[:, i * XCOL:(i + 1) * XCOL],
                                           Eall[:, r * NC:r * NC + XCOL])
                cp.wait_op(s_vd, n_chain * (r - 2) + 1, "sem-ge")
                cp.then_inc(s_st, 1)

    return nc


def _simulate_device(xp_core, plan):
    """Numpy emulation of the per-core program (bf16 rounding)."""
    chains = plan["chains"]
    NC = sum(chains)
    bf = ml_dtypes.bfloat16
    wr = np.asarray(xp_core[:, 0:128], np.float32)
    X = np.asarray(xp_core[:, 128:], np.float32).reshape(128, NR, NC)
    Ee = np.ones((L, 128, NC), bf)
    off = np.cumsum([0] + chains)
    Ee[1] = X[:, 0, :]
    for r in range(2, RL + 1):
        psv = wr.T @ np.asarray(Ee[r - 1], np.float32)
        # routed chains rematerialize ps through bf16 before the mul
        psv[:, off[2]:] = psv[:, off[2]:].astype(bf).astype(np.float32)
        Ee[r] = (psv * X[:, r - 1, :]).astype(bf)
    snaps = np.concatenate([Ee[RC[0]], Ee[RC[1]], Ee[RP[0]], Ee[RP[1]]],
                           axis=1).astype(bf)
    stage = np.concatenate(
        [Ee[r][:, 0:XCOL] for r in range(R_EX0, R_EX1 + 1)],
        axis=1).astype(bf)
    return snaps, stage


def _postprocess(snaps, stage, plan, core, c):
    """Per-core host math (float64): stitch offsets, read finals."""
    chains = plan["chains"]
    NC = sum(chains)
    off = np.cumsum([0] + chains)
    place = plan["placements"][core]

    lsn = np.log(np.maximum(np.asarray(snaps, np.float64), 1e-300))
    lst = np.log(np.maximum(np.asarray(stage, np.float64), 1e-300))

    def cell_snap(cell, k):
        # k: 0,1 -> RC rounds; 2,3 -> RP rounds
        cc, jj, hh = place[cell]
        return lsn[64 * hh:64 * hh + 64, k * NC + off[cc] + jj]

    A0 = -(W + 1) * c
    res = {}
    for (s_ex, b, r_ex) in plan["extr"][core]:
        A = A0
        for s in range(1, s_ex + 1):
            d = 0.0
            for k in (0, 1):
                prev = cell_snap((s - 1, b), 2 + k)
                cur = cell_snap((s, b), k)
                d += ((prev + RP[k] * c) - (cur + RC[k] * c)).mean()
            A += d / 2
        cc, jj, hh = place[(s_ex, b)]
        assert cc == 0 and jj < XCOL
        lf = lst[64 * hh:64 * hh + 64, (r_ex - R_EX0) * XCOL + jj]
        res[b] = lf.sum() + 64.0 * (r_ex * c + A)
    return res


def kernel(pad_x, transitions, origination, batch_sizes, _simulate=False):
    pad_x = np.asarray(pad_x)
    transitions = np.asarray(transitions)
    origination = np.asarray(origination)
    batch_sizes = np.asarray(batch_sizes)

    c = _c_step(transitions, pad_x)
    plan = _plan(batch_sizes)
    xp, NC = _build_host_inputs(pad_x, transitions, origination, c, plan)

    if _simulate:
        outs = [_simulate_device(xp[i], plan) for i in range(NCORES)]
    else:
        from concourse.bass_utils import run_bass_kernel_spmd
        key = (batch_sizes.tobytes(), W, SL)
        if key not in _CACHE:
            _CACHE[key] = _build_program(plan)
        nc = _CACHE[key]
        in_maps = [{"xp": xp[i]} for i in range(NCORES)]
        out = run_bass_kernel_spmd(nc, in_maps, list(range(NCORES)))
        outs = [(out.results[i]["snaps"], out.results[i]["stage"])
                for i in range(NCORES)]

    res = np.empty(B, np.float32)
    for i in range(NCORES):
        snaps, stage = outs[i]
        for b, v in _postprocess(snaps, stage, plan, i, c).items():
            res[b] = v
    return res


# revision 23
# speedup vs baseline: 2.7908x; 1.0420x over previous
"""Linear-chain CRF forward pass on 8 Trainium2 NeuronCores.

Reference recurrence (per batch element b):
    alpha_t[j] = x_t[j] + logsumexp_k(alpha_{t-1}[k] + trans[j,k])
    out[b] = sum_j alpha_{L_b - 1}[j]

Device formulation: exp space with a constant per-step log shift c folded
into the transition matrix:
    E_r = (Mc @ E_{r-1}) * X_r,  Mc[j,k] = exp(trans[j,k] - c)
so alpha_{t(r)} = log E_r + r*c + A for a per-cell constant A.

The 2048-step serial chain is broken via the Birkhoff contraction of the
positive map E -> Mc @ E: time is cut into segments of SL=16 steps; each
(segment, batch) pair is a "cell" (64 classes) warmed up W rounds from an
all-ones init.  Only VALID cells (t < L_b + warmup margin) are packed, so
length padding costs neither compute nor DMA.  The unknown per-cell
offsets A are recovered on the host by telescoping mean log-ratios at
shared timesteps (two per boundary, averaged), read from snapshot DMAs of
the write-once E history at rounds {RC} (segment side) / {RP}
(predecessor side).

Per-core layout: cells pack 2 per column (row blocks 0:64 / 64:128).
Columns are split across 4 independent chains: elementwise muls for 2
chains on DVE and 2 on Pool (GPSIMD), all matmuls on PE (block-diagonal
bf16 lhsT), extraction staging copies on ACT.  Every cross-engine dep is
an embedded single wait (wait_op); X arrival uses per-chunk standalone
waits.  Segment 0 replays the exact trajectory: its round-(W+1) X column
is exp(alpha_0) / (Mc^{W+1} @ 1) so E[W+1] == exp(alpha_0) exactly.
Cells whose extraction would land on the last two rounds get a phantom
successor cell extracted during its warmup instead, keeping extraction
<= round W+PH_M so the final staging DMA overlaps the tail rounds.
"""

from contextlib import ExitStack

import numpy as np
import ml_dtypes

B, T, C = 256, 2048, 64
NRT = 1                    # routed (ACT-copy + DVE-2x) chains
NCORES = 8
BPC = B // NCORES          # 32
XCOL = BPC // 2            # extraction columns in chain 0 (16)

_CACHE = {}


def set_config(w=6, sl=16, rc0=3, ph_m=13, n_st=2):
    """(Re)derive the round-structure constants.  Rounds r = 2..L-1 execute
    on device (round 1 is folded into X on the host: E[1] == X'[1]).
    n_st: stitch rounds averaged per segment boundary (1 or 2)."""
    g = globals()
    g["W"], g["SL"] = w, sl
    g["L"] = sl + w + 1
    g["NR"] = g["L"] - 1
    g["N_ST"] = n_st
    g["RC"] = tuple(rc0 + i for i in range(n_st))   # stitch, current segment
    g["RP"] = tuple(rc0 + sl + i for i in range(n_st))  # stitch, predecessor
    g["PH_M"] = ph_m                  # m >= PH_M -> phantom successor cell
    g["R_EX0"] = ph_m - sl + w + 1    # earliest extraction round
    g["R_EX1"] = ph_m + w             # latest extraction round (m+W+1)
    g["N_EX"] = g["R_EX1"] - g["R_EX0"] + 1
    # rounds beyond every consumer (stitch service, extraction) are dead
    g["RL"] = max(g["RP"][-1], g["R_EX1"])
    assert g["R_EX0"] >= 2 and g["RL"] <= g["L"] - 1, (w, sl, rc0, ph_m)


set_config(w=7, rc0=1, ph_m=10, n_st=1)


def _c_step(transitions, pad_x):
    """Mean per-step growth of max_j alpha, from a short host simulation."""
    x = np.asarray(pad_x[:4], np.float64)
    tr = np.asarray(transitions, np.float64)
    a = x[:, 0, :]
    tot, n = 0.0, 0
    for t in range(1, 257):
        s = a[:, None, :] + tr[None, :, :]
        m = s.max(axis=2, keepdims=True)
        a_new = x[:, t, :] + np.log(np.exp(s - m).sum(axis=2)) + m[:, :, 0]
        tot += float((a_new.max(axis=1) - a.max(axis=1)).mean())
        n += 1
        a = a_new
    return tot / n


def _plan(batch_sizes):
    """Cell enumeration + column assignment, shared across cores."""
    bs = np.asarray(batch_sizes, np.int64)
    cores = [[] for _ in range(NCORES)]          # all cells per core
    extr = [[] for _ in range(NCORES)]           # (s_ex, b, r_ex) per core
    for b in range(B):
        core = b % NCORES
        t_star = int(bs[b]) - 1
        s_max, m = divmod(t_star, SL)
        phantom = m >= PH_M
        for s in range(s_max + 1 + (1 if phantom else 0)):
            cores[core].append((s, b))
        if phantom:
            extr[core].append((s_max + 1, b, m - SL + W + 1))
        else:
            extr[core].append((s_max, b, m + W + 1))
    ncells = max(len(c) for c in cores)
    tc = XCOL + (max(0, ncells - 2 * XCOL) + 1) // 2   # total columns
    tc = max(tc, XCOL + 4)
    # 2 direct DVE chains + NRT routed chains (ACT copies PSUM->SBUF bf16,
    # then DVE multiplies in 2x mode).  GPSIMD cannot read PSUM and ACT
    # cannot run TensorTensor, so ACT's idle copy bandwidth is the only way
    # to feed a second mul path.  Calibrated round model:
    #   DVE busy = 2*(125+1.0417*Nd) + NRT*(60+0.52*Nr)
    #   ACT busy = NRT*(185+0.833*Nr);  routed latency = 792+1.353*Nr
    best = None
    for nr in range(0, tc // 2 + 1):
        nd = (tc - NRT * nr + 1) // 2
        if nd < XCOL or (NRT * nr and nr < 8):
            continue
        dve = 2 * (125 + 1.0417 * nd) + NRT * (60 + 0.52 * nr)
        act = NRT * (185 + 0.833 * nr)
        r = max(dve, act, 518 + 1.0417 * nd, (792 + 1.353 * nr) if nr else 0)
        if best is None or r < best[0]:
            best = (r, nd, nr)
    _, nd, nr = best
    chains = [nd, tc - NRT * nr - nd] + [nr] * (NRT if nr else 0)
    n_chain = len(chains)

    placements = []
    for core in range(NCORES):
        place = {}
        ex = sorted(extr[core], key=lambda e: e[1])   # by global b
        assert len(ex) == BPC
        ex_set = set()
        for slot, (s, b, r_ex) in enumerate(ex):
            place[(s, b)] = (0, slot // 2, slot % 2)
            ex_set.add((s, b))
        rest = [cell for cell in cores[core] if cell not in ex_set]
        free = [(0, j) for j in range(XCOL, chains[0])]
        for cc in range(1, n_chain):
            free += [(cc, j) for j in range(chains[cc])]
        slots = [(cc, jj, hh) for (cc, jj) in free for hh in (0, 1)]
        assert len(rest) <= len(slots), (len(rest), len(slots))
        for cell, sl_ in zip(rest, slots):
            place[cell] = sl_
        placements.append(place)
    return dict(chains=chains, placements=placements, extr=extr)


def _build_host_inputs(pad_x, transitions, origination, c, plan):
    """xp per core: [128, 128 + NR*NC] bf16 = [wmat | X rounds 1..NR]."""
    chains = plan["chains"]
    NC = sum(chains)
    off = np.cumsum([0] + chains)

    mc = np.exp(np.asarray(transitions, np.float64) - c)
    wmat = np.zeros((128, 128), np.float64)
    wmat[:64, :64] = mc.T
    wmat[64:, 64:] = mc.T

    u1 = mc @ np.ones(64, np.float64)      # baked into X'[1] == E[1]
    u = np.ones(64, np.float64)            # Mc^{W+1} @ 1
    for _ in range(W + 1):
        u = mc @ u

    x = np.asarray(pad_x, np.float64)
    alpha0 = x[:, 0, :] + np.asarray(origination, np.float64)[None, :]

    xp = np.ones((NCORES, 128, 128 + NR * NC), np.float64)
    xp[:, :, :128] = wmat[None]
    for core in range(NCORES):
        for (s, b), (cc, jj, hh) in plan["placements"][core].items():
            col = 128 + np.arange(NR) * NC + off[cc] + jj
            rows = slice(64 * hh, 64 * hh + 64)
            t0 = SL * s - W - 1
            ts = t0 + np.arange(1, L)
            xs = np.ones((NR, 64))
            vt = (ts >= (1 if s == 0 else 0)) & (ts < T)
            xs[vt] = np.exp(x[b, ts[vt]])
            if s == 0:
                xs[W] = np.exp(alpha0[b]) / u
            xs[0] = xs[0] * u1
            # mixed advanced/basic indexing puts the col axis first
            xp[core, rows, col] = xs
    return xp.astype(ml_dtypes.bfloat16), NC


def _build_program(plan):
    import concourse.bass as bass
    from concourse import mybir

    dt = mybir.dt
    chains = plan["chains"]
    n_chain = len(chains)
    n_rt = n_chain - 2                   # routed chains (ACT copy + DVE 2x)
    NC = sum(chains)
    off = np.cumsum([0] + chains)
    CHUNKS = [(1, 1), (2, 2), (3, 3), (4, 5), (6, 8), (9, 12), (13, 16),
              (17, RL)]
    N_EARLY = 3                     # chunks issued in the entry BB
    NSX = 3

    nc = bass.Bass()
    xp = nc.declare_dram_parameter("xp", [128, 128 + NR * NC],
                                   dt.bfloat16, False)
    snaps = nc.declare_dram_parameter("snaps", [128, 4 * NC], dt.bfloat16,
                                      True)
    stage_o = nc.declare_dram_parameter("stage", [128, N_EX * XCOL],
                                        dt.bfloat16, True)

    with ExitStack() as ctx:
        def sb(name, shape, d):
            return ctx.enter_context(nc.sbuf_tensor(name, shape, d))
        xbuf = sb("xbuf", [128, 128 + NR * NC], dt.bfloat16)
        wr = xbuf[:, 0:128]
        Eall = sb("eall", [128, (RL + 1) * NC], dt.bfloat16)
        stage = sb("stage_t", [128, N_EX * XCOL], dt.bfloat16)
        # write-once SBUF copies of routed chains' matmul results
        psb = [sb(f"psb{j}", [128, (RL + 1) * chains[2 + j]], dt.bfloat16)
               for j in range(n_rt)]
        ps = [ctx.enter_context(nc.psum_tensor(f"ps{cc}", [128, n],
                                               dt.float32))
              for cc, n in enumerate(chains)]
        s_x = [ctx.enter_context(nc.semaphore(f"s_x{i}")) for i in range(NSX)]
        s_pe = ctx.enter_context(nc.semaphore("s_pe"))
        s_vd = ctx.enter_context(nc.semaphore("s_vd"))
        s_ac = ctx.enter_context(nc.semaphore("s_ac"))
        s_st = ctx.enter_context(nc.semaphore("s_st"))
        s_o = ctx.enter_context(nc.semaphore("s_o"))

        def esl(cc, r):
            base = r * NC + off[cc]
            return Eall[:, base:base + chains[cc]]

        def psl(j, r):
            n = chains[2 + j]
            return psb[j][:, r * n:(r + 1) * n]

        def xsl(cc, r):
            base = 128 + (r - 1) * NC + off[cc]
            return xbuf[:, base:base + chains[cc]]

        def chunk_of(r):
            for k, (r0, r1) in enumerate(CHUNKS):
                if r0 <= r <= r1:
                    return k
            raise AssertionError(r)

        def xtarget(k):
            return 16 * (k // NSX + 1)

        def issue_chunk(eng, k):
            r0, r1 = CHUNKS[k]
            a = 0 if k == 0 else 128 + (r0 - 1) * NC
            bnd = 128 + r1 * NC
            d = eng.dma_start(xbuf[:, a:bnd], xp[:, a:bnd])
            if k >= NSX:
                d.wait_op(s_x[k % NSX], 16 * (k // NSX), "sem-ge")
            d.then_inc(s_x[k % NSX], 16)

        for k in range(N_EARLY):
            issue_chunk(nc.sync, k)

        block = ctx.enter_context(nc.Block())

        @block.sync
        def _(sync):
            for k in range(N_EARLY, len(CHUNKS)):
                issue_chunk(sync, k)
            # stitch snapshots (predecessor side) + extraction staging out
            for k in range(N_ST):
                sync.wait_ge(s_vd, n_chain * (RP[k] - 1))
                sync.dma_start(snaps[:, (2 + k) * NC:(3 + k) * NC],
                               Eall[:, RP[k] * NC:(RP[k] + 1) * NC]
                               ).then_inc(s_o, 16)
            sync.dma_start(stage_o[:, :], stage[:]).wait_op(
                s_st, N_EX, "sem-ge").then_inc(s_o, 16)

        @block.tensor
        def _(tensor):
            # p-state warmup: PE ramps to full clock after 3us of activity
            for _ in range(7):
                nc.tensor.matmul(ps[0][:], Eall[:, 0:128].bitcast(
                    mybir.dt.bfloat16), Eall[:, 0:chains[0]],
                    start=True, stop=True)
            for r in range(2, RL + 1):
                for cc in range(n_chain):
                    rhs = xsl(cc, 1) if r == 2 else esl(cc, r - 1)
                    mm = nc.tensor.matmul(ps[cc][:], wr, rhs,
                                          start=True, stop=True)
                    if r == 2:
                        mm.wait_op(s_x[0], 16, "sem-ge")
                    else:
                        # mul(r-1) done implies ps[cc] free and E[r-1] ready
                        mm.wait_op(s_vd, n_chain * (r - 3) + cc + 1, "sem-ge")
                    mm.then_inc(s_pe, 1)

        @block.scalar
        def _(scalar):
            # PSUM -> SBUF bf16 rematerialization feeding the DVE 2x muls
            for r in range(2, RL + 1):
                for j in range(n_rt):
                    cp = nc.scalar.copy(psl(j, r), ps[2 + j][:])
                    cp.wait_op(s_pe, n_chain * (r - 2) + 2 + j + 1, "sem-ge")
                    cp.then_inc(s_ac, 1)

        @block.vector
        def _(vector):
            last_k = -1
            for r in range(2, RL + 1):
                k = chunk_of(r)
                if k != last_k:
                    vector.wait_ge(s_x[k % NSX], xtarget(k))
                    last_k = k
                for cc in range(n_chain):
                    if cc < 2:
                        t = nc.vector.tensor_mul(esl(cc, r), ps[cc][:],
                                                 xsl(cc, r))
                        t.wait_op(s_pe, n_chain * (r - 2) + cc + 1, "sem-ge")
                    else:
                        t = nc.vector.tensor_mul(esl(cc, r), psl(cc - 2, r),
                                                 xsl(cc, r))
                        t.wait_op(s_ac, n_rt * (r - 2) + (cc - 2) + 1,
                                  "sem-ge")
                    t.then_inc(s_vd, 1)

        @block.gpsimd
        def _(gpsimd):
            # stitch snapshot (segment side) + extraction staging copies
            for k in range(N_ST):
                if RC[k] == 1:
                    # round 1 is E[1] == X'[1], resident in xbuf, not Eall
                    gpsimd.wait_ge(s_x[0], 16)
                    gpsimd.dma_start(snaps[:, k * NC:(k + 1) * NC],
                                     xbuf[:, 128:128 + NC]
                                     ).then_inc(s_o, 16)
                else:
                    gpsimd.wait_ge(s_vd, n_chain * (RC[k] - 1))
                    gpsimd.dma_start(snaps[:, k * NC:(k + 1) * NC],
                                     Eall[:, RC[k] * NC:(RC[k] + 1) * NC]
                                     ).then_inc(s_o, 16)
            for i, r in enumerate(range(R_EX0, R_EX1 + 1)):
                cp = nc.gpsimd.tensor_copy(stage[:, i * XCOL:(i + 1) * XCOL],
                                           Eall[:, r * NC:r * NC + XCOL])
                cp.wait_op(s_vd, n_chain * (r - 2) + 1, "sem-ge")
                cp.then_inc(s_st, 1)

    return nc


def _simulate_device(xp_core, plan):
    """Numpy emulation of the per-core program (bf16 rounding)."""
    chains = plan["chains"]
    NC = sum(chains)
    bf = ml_dtypes.bfloat16
    wr = np.asarray(xp_core[:, 0:128], np.float32)
    X = np.asarray(xp_core[:, 128:], np.float32).reshape(128, NR, NC)
    Ee = np.ones((L, 128, NC), bf)
    off = np.cumsum([0] + chains)
    Ee[1] = X[:, 0, :]
    for r in range(2, RL + 1):
        psv = wr.T @ np.asarray(Ee[r - 1], np.float32)
        # routed chains rematerialize ps through bf16 before the mul
        psv[:, off[2]:] = psv[:, off[2]:].astype(bf).astype(np.float32)
        Ee[r] = (psv * X[:, r - 1, :]).astype(bf)
    rc1 = Ee[RC[1]] if N_ST > 1 else np.zeros_like(Ee[0])
    rp1 = Ee[RP[1]] if N_ST > 1 else np.zeros_like(Ee[0])
    snaps = np.concatenate([Ee[RC[0]], rc1, Ee[RP[0]], rp1],
                           axis=1).astype(bf)
    stage = np.concatenate(
        [Ee[r][:, 0:XCOL] for r in range(R_EX0, R_EX1 + 1)],
        axis=1).astype(bf)
    return snaps, stage


def _postprocess(snaps, stage, plan, core, c):
    """Per-core host math (float64): stitch offsets, read finals."""
    chains = plan["chains"]
    NC = sum(chains)
    off = np.cumsum([0] + chains)
    place = plan["placements"][core]

    lsn = np.log(np.maximum(np.asarray(snaps, np.float64), 1e-300))
    lst = np.log(np.maximum(np.asarray(stage, np.float64), 1e-300))

    def cell_snap(cell, k):
        # k: 0,1 -> RC rounds; 2,3 -> RP rounds
        cc, jj, hh = place[cell]
        return lsn[64 * hh:64 * hh + 64, k * NC + off[cc] + jj]

    A0 = -(W + 1) * c
    res = {}
    for (s_ex, b, r_ex) in plan["extr"][core]:
        A = A0
        for s in range(1, s_ex + 1):
            d = 0.0
            for k in range(N_ST):
                prev = cell_snap((s - 1, b), 2 + k)
                cur = cell_snap((s, b), k)
                d += ((prev + RP[k] * c) - (cur + RC[k] * c)).mean()
            A += d / N_ST
        cc, jj, hh = place[(s_ex, b)]
        assert cc == 0 and jj < XCOL
        lf = lst[64 * hh:64 * hh + 64, (r_ex - R_EX0) * XCOL + jj]
        res[b] = lf.sum() + 64.0 * (r_ex * c + A)
    return res


def kernel(pad_x, transitions, origination, batch_sizes, _simulate=False):
    pad_x = np.asarray(pad_x)
    transitions = np.asarray(transitions)
    origination = np.asarray(origination)
    batch_sizes = np.asarray(batch_sizes)

    c = _c_step(transitions, pad_x)
    plan = _plan(batch_sizes)
    xp, NC = _build_host_inputs(pad_x, transitions, origination, c, plan)

    if _simulate:
        outs = [_simulate_device(xp[i], plan) for i in range(NCORES)]
    else:
        from concourse.bass_utils import run_bass_kernel_spmd
        key = (batch_sizes.tobytes(), W, SL)
        if key not in _CACHE:
            _CACHE[key] = _build_program(plan)
        nc = _CACHE[key]
        in_maps = [{"xp": xp[i]} for i in range(NCORES)]
        out = run_bass_kernel_spmd(nc, in_maps, list(range(NCORES)))
        outs = [(out.results[i]["snaps"], out.results[i]["stage"])
                for i in range(NCORES)]

    res = np.empty(B, np.float32)
    for i in range(NCORES):
        snaps, stage = outs[i]
        for b, v in _postprocess(snaps, stage, plan, i, c).items():
            res[b] = v
    return res


# revision 29
# speedup vs baseline: 2.8627x; 1.0258x over previous
"""Linear-chain CRF forward pass on 8 Trainium2 NeuronCores.

Reference recurrence (per batch element b):
    alpha_t[j] = x_t[j] + logsumexp_k(alpha_{t-1}[k] + trans[j,k])
    out[b] = sum_j alpha_{L_b - 1}[j]

Device formulation: exp space with a constant per-step log shift c folded
into the transition matrix:
    E_r = (Mc @ E_{r-1}) * X_r,  Mc[j,k] = exp(trans[j,k] - c)
so alpha_{t(r)} = log E_r + r*c + A for a per-cell constant A.

The 2048-step serial chain is broken via the Birkhoff contraction of the
positive map E -> Mc @ E: time is cut into segments of SL=16 steps; each
(segment, batch) pair is a "cell" (64 classes) warmed up W rounds from an
all-ones init.  Only VALID cells (t < L_b + warmup margin) are packed, so
length padding costs neither compute nor DMA.  The unknown per-cell
offsets A are recovered on the host by telescoping mean log-ratios at
shared timesteps (two per boundary, averaged), read from snapshot DMAs of
the write-once E history at rounds {RC} (segment side) / {RP}
(predecessor side).

Per-core layout: cells pack 2 per column (row blocks 0:64 / 64:128).
Columns are split across 4 independent chains: elementwise muls for 2
chains on DVE and 2 on Pool (GPSIMD), all matmuls on PE (block-diagonal
bf16 lhsT), extraction staging copies on ACT.  Every cross-engine dep is
an embedded single wait (wait_op); X arrival uses per-chunk standalone
waits.  Segment 0 replays the exact trajectory: its round-(W+1) X column
is exp(alpha_0) / (Mc^{W+1} @ 1) so E[W+1] == exp(alpha_0) exactly.
Cells whose extraction would land on the last two rounds get a phantom
successor cell extracted during its warmup instead, keeping extraction
<= round W+PH_M so the final staging DMA overlaps the tail rounds.
"""

from contextlib import ExitStack

import numpy as np
import ml_dtypes

B, T, C = 256, 2048, 64
NRT = 1                    # routed (ACT-copy + DVE-2x) chains
NCORES = 8
BPC = B // NCORES          # 32
XCOL = BPC // 2            # extraction columns in chain 0 (16)

_CACHE = {}


def set_config(w=6, sl=16, rc0=3, ph_m=13, n_st=2):
    """(Re)derive the round-structure constants.  Rounds r = 2..L-1 execute
    on device (round 1 is folded into X on the host: E[1] == X'[1]).
    n_st: stitch rounds averaged per segment boundary (1 or 2)."""
    g = globals()
    g["W"], g["SL"] = w, sl
    g["L"] = sl + w + 1
    g["NR"] = g["L"] - 1
    g["N_ST"] = n_st
    g["RC"] = tuple(rc0 + i for i in range(n_st))   # stitch, current segment
    g["RP"] = tuple(rc0 + sl + i for i in range(n_st))  # stitch, predecessor
    g["PH_M"] = ph_m                  # m >= PH_M -> phantom successor cell
    g["R_EX0"] = ph_m - sl + w + 1    # earliest extraction round
    g["R_EX1"] = ph_m + w             # latest extraction round (m+W+1)
    g["N_EX"] = g["R_EX1"] - g["R_EX0"] + 1
    # rounds beyond every consumer (stitch service, extraction) are dead
    g["RL"] = max(g["RP"][-1], g["R_EX1"])
    assert g["R_EX0"] >= 2 and g["RL"] <= g["L"] - 1, (w, sl, rc0, ph_m)


set_config(w=7, rc0=1, ph_m=10, n_st=1)


def _c_step(transitions, pad_x):
    """Mean per-step growth of max_j alpha, from a short host simulation."""
    x = np.asarray(pad_x[:4], np.float64)
    tr = np.asarray(transitions, np.float64)
    a = x[:, 0, :]
    tot, n = 0.0, 0
    for t in range(1, 257):
        s = a[:, None, :] + tr[None, :, :]
        m = s.max(axis=2, keepdims=True)
        a_new = x[:, t, :] + np.log(np.exp(s - m).sum(axis=2)) + m[:, :, 0]
        tot += float((a_new.max(axis=1) - a.max(axis=1)).mean())
        n += 1
        a = a_new
    return tot / n


def _plan(batch_sizes):
    """Cell enumeration + column assignment, shared across cores."""
    bs = np.asarray(batch_sizes, np.int64)
    cores = [[] for _ in range(NCORES)]          # all cells per core
    extr = [[] for _ in range(NCORES)]           # (s_ex, b, r_ex) per core
    for b in range(B):
        core = b % NCORES
        t_star = int(bs[b]) - 1
        s_max, m = divmod(t_star, SL)
        phantom = m >= PH_M
        for s in range(s_max + 1 + (1 if phantom else 0)):
            cores[core].append((s, b))
        if phantom:
            extr[core].append((s_max + 1, b, m - SL + W + 1))
        else:
            extr[core].append((s_max, b, m + W + 1))
    ncells = max(len(c) for c in cores)
    tc = XCOL + (max(0, ncells - 2 * XCOL) + 1) // 2   # total columns
    tc = max(tc, XCOL + 4)
    # 2 direct DVE chains + NRT routed chains (ACT copies PSUM->SBUF bf16,
    # then DVE multiplies in 2x mode).  GPSIMD cannot read PSUM and ACT
    # cannot run TensorTensor, so ACT's idle copy bandwidth is the only way
    # to feed a second mul path.  Calibrated round model:
    #   DVE busy = 2*(125+1.0417*Nd) + NRT*(60+0.52*Nr)
    #   ACT busy = NRT*(185+0.833*Nr);  routed latency = 792+1.353*Nr
    best = None
    for nr in range(0, tc // 2 + 1):
        nd = (tc - NRT * nr + 1) // 2
        if nd < XCOL or (NRT * nr and nr < 8):
            continue
        dve = 2 * (125 + 1.0417 * nd) + NRT * (60 + 0.52 * nr)
        act = NRT * (185 + 0.833 * nr)
        r = max(dve, act, 518 + 1.0417 * nd, (792 + 1.353 * nr) if nr else 0)
        if best is None or r < best[0]:
            best = (r, nd, nr)
    _, nd, nr = best
    chains = [nd, tc - NRT * nr - nd] + [nr] * (NRT if nr else 0)
    if sum(chains) % 2:
        chains[1] += 1        # keep NC even for the fp8->bf16 bitcast
    n_chain = len(chains)

    placements = []
    for core in range(NCORES):
        place = {}
        ex = sorted(extr[core], key=lambda e: e[1])   # by global b
        assert len(ex) == BPC
        ex_set = set()
        for slot, (s, b, r_ex) in enumerate(ex):
            place[(s, b)] = (0, slot // 2, slot % 2)
            ex_set.add((s, b))
        rest = [cell for cell in cores[core] if cell not in ex_set]
        free = [(0, j) for j in range(XCOL, chains[0])]
        for cc in range(1, n_chain):
            free += [(cc, j) for j in range(chains[cc])]
        slots = [(cc, jj, hh) for (cc, jj) in free for hh in (0, 1)]
        assert len(rest) <= len(slots), (len(rest), len(slots))
        for cell, sl_ in zip(rest, slots):
            place[cell] = sl_
        placements.append(place)
    return dict(chains=chains, placements=placements, extr=extr)


def _build_host_inputs(pad_x, transitions, origination, c, plan):
    """xp per core: [128, 128 + NR*NC] bf16 = [wmat | X rounds 1..NR]."""
    chains = plan["chains"]
    NC = sum(chains)
    off = np.cumsum([0] + chains)

    mc = np.exp(np.asarray(transitions, np.float64) - c)
    wmat = np.zeros((128, 128), np.float64)
    wmat[:64, :64] = mc.T
    wmat[64:, 64:] = mc.T

    u1 = mc @ np.ones(64, np.float64)      # baked into X'[1] == E[1]
    u = np.ones(64, np.float64)            # Mc^{W+1} @ 1
    for _ in range(W + 1):
        u = mc @ u

    x = np.asarray(pad_x, np.float64)
    alpha0 = x[:, 0, :] + np.asarray(origination, np.float64)[None, :]

    xp = np.ones((NCORES, 128, 128 + NR * NC), np.float64)
    xp[:, :, :128] = wmat[None]
    for core in range(NCORES):
        for (s, b), (cc, jj, hh) in plan["placements"][core].items():
            col = 128 + np.arange(NR) * NC + off[cc] + jj
            rows = slice(64 * hh, 64 * hh + 64)
            t0 = SL * s - W - 1
            ts = t0 + np.arange(1, L)
            xs = np.ones((NR, 64))
            vt = (ts >= (1 if s == 0 else 0)) & (ts < T)
            xs[vt] = np.exp(x[b, ts[vt]])
            if s == 0:
                xs[W] = np.exp(alpha0[b]) / u
            xs[0] = xs[0] * u1
            # mixed advanced/basic indexing puts the col axis first
            xp[core, rows, col] = xs
    xb = xp.astype(ml_dtypes.bfloat16)
    xp0 = np.empty((NCORES, 128, 256 + NC), ml_dtypes.float8_e4m3)
    xp0[:, :, :256] = xb[:, :, :128].view(np.uint8).view(ml_dtypes.float8_e4m3)
    xp0[:, :, 256:] = xp[:, :, 128:128 + NC]
    return (xp0, xb[:, :, 128 + NC:]), NC


def _build_program(plan):
    import concourse.bass as bass
    from concourse import mybir

    dt = mybir.dt
    chains = plan["chains"]
    n_chain = len(chains)
    n_rt = n_chain - 2                   # routed chains (ACT copy + DVE 2x)
    NC = sum(chains)
    off = np.cumsum([0] + chains)
    CHUNKS = [(1, 1), (2, 2), (3, 3), (4, 5), (6, 8), (9, 12), (13, 16),
              (17, RL)]
    N_EARLY = 3                     # chunks issued in the entry BB
    NSX = 3

    nc = bass.Bass()
    xp0 = nc.declare_dram_parameter("xp0", [128, 256 + NC], dt.float8e4,
                                    False)
    xp = nc.declare_dram_parameter("xp", [128, (NR - 1) * NC],
                                   dt.bfloat16, False)
    snaps = nc.declare_dram_parameter("snaps", [128, 4 * NC], dt.bfloat16,
                                      True)
    stage_o = nc.declare_dram_parameter("stage", [128, N_EX * XCOL],
                                        dt.bfloat16, True)
    snap1 = nc.declare_dram_parameter("snap1", [128, NC], dt.float8e4, True)

    with ExitStack() as ctx:
        def sb(name, shape, d):
            return ctx.enter_context(nc.sbuf_tensor(name, shape, d))
        x0buf = sb("x0buf", [128, 256 + NC], dt.float8e4)
        xbuf = sb("xbuf", [128, (NR - 1) * NC], dt.bfloat16)
        wr = x0buf[:, 0:256].bitcast(dt.bfloat16)
        Eall = sb("eall", [128, (RL + 1) * NC], dt.bfloat16)
        stage = sb("stage_t", [128, N_EX * XCOL], dt.bfloat16)
        # write-once SBUF copies of routed chains' matmul results
        psb = [sb(f"psb{j}", [128, (RL + 1) * chains[2 + j]], dt.bfloat16)
               for j in range(n_rt)]
        ps = [ctx.enter_context(nc.psum_tensor(f"ps{cc}", [128, n],
                                               dt.float32))
              for cc, n in enumerate(chains)]
        s_x = [ctx.enter_context(nc.semaphore(f"s_x{i}")) for i in range(NSX)]
        s_pe = ctx.enter_context(nc.semaphore("s_pe"))
        s_vd = ctx.enter_context(nc.semaphore("s_vd"))
        s_ac = ctx.enter_context(nc.semaphore("s_ac"))
        s_st = ctx.enter_context(nc.semaphore("s_st"))
        s_o = ctx.enter_context(nc.semaphore("s_o"))

        def esl(cc, r):
            base = r * NC + off[cc]
            return Eall[:, base:base + chains[cc]]

        def psl(j, r):
            n = chains[2 + j]
            return psb[j][:, r * n:(r + 1) * n]

        def xsl(cc, r):
            if r == 1:
                base = 256 + off[cc]
                return x0buf[:, base:base + chains[cc]]
            base = (r - 2) * NC + off[cc]
            return xbuf[:, base:base + chains[cc]]

        def chunk_of(r):
            for k, (r0, r1) in enumerate(CHUNKS):
                if r0 <= r <= r1:
                    return k
            raise AssertionError(r)

        def xtarget(k):
            return 16 * (k // NSX + 1)

        def issue_chunk(eng, k):
            if k == 0:
                d = eng.dma_start(x0buf[:], xp0[:, :])
            else:
                r0, r1 = CHUNKS[k]
                a, bnd = (r0 - 2) * NC, (r1 - 1) * NC
                d = eng.dma_start(xbuf[:, a:bnd], xp[:, a:bnd])
                if k >= NSX:
                    d.wait_op(s_x[k % NSX], 16 * (k // NSX), "sem-ge")
            d.then_inc(s_x[k % NSX], 16)

        for k in range(N_EARLY):
            issue_chunk(nc.sync, k)

        block = ctx.enter_context(nc.Block())

        @block.sync
        def _(sync):
            for k in range(N_EARLY, len(CHUNKS)):
                issue_chunk(sync, k)
            # stitch snapshots (predecessor side) + extraction staging out
            for k in range(N_ST):
                sync.wait_ge(s_vd, n_chain * (RP[k] - 1))
                sync.dma_start(snaps[:, (2 + k) * NC:(3 + k) * NC],
                               Eall[:, RP[k] * NC:(RP[k] + 1) * NC]
                               ).then_inc(s_o, 16)
            sync.dma_start(stage_o[:, :], stage[:]).wait_op(
                s_st, N_EX, "sem-ge").then_inc(s_o, 16)

        @block.tensor
        def _(tensor):
            # p-state warmup: PE ramps to full clock after 3us of
            # continuous activity; fill the preamble-to-first-X window with
            # small dummy matmuls so real rounds start at full speed
            for _ in range(26):
                nc.tensor.matmul(ps[0][:, 0:128], Eall[:, 0:128].bitcast(
                    mybir.dt.bfloat16), Eall[:, 0:128],
                    start=True, stop=True)
            for r in range(2, RL + 1):
                for cc in range(n_chain):
                    rhs = xsl(cc, 1) if r == 2 else esl(cc, r - 1)
                    mm = nc.tensor.matmul(ps[cc][:], wr, rhs,
                                          start=True, stop=True)
                    if r == 2:
                        mm.wait_op(s_x[0], 16, "sem-ge")
                    else:
                        # mul(r-1) done implies ps[cc] free and E[r-1] ready
                        mm.wait_op(s_vd, n_chain * (r - 3) + cc + 1, "sem-ge")
                    mm.then_inc(s_pe, 1)

        @block.scalar
        def _(scalar):
            # PSUM -> SBUF bf16 rematerialization feeding the DVE 2x muls
            for r in range(2, RL + 1):
                for j in range(n_rt):
                    cp = nc.scalar.copy(psl(j, r), ps[2 + j][:])
                    cp.wait_op(s_pe, n_chain * (r - 2) + 2 + j + 1, "sem-ge")
                    cp.then_inc(s_ac, 1)

        @block.vector
        def _(vector):
            last_k = -1
            for r in range(2, RL + 1):
                k = chunk_of(r)
                if k != last_k:
                    vector.wait_ge(s_x[k % NSX], xtarget(k))
                    last_k = k
                for cc in range(n_chain):
                    if cc < 2:
                        t = nc.vector.tensor_mul(esl(cc, r), ps[cc][:],
                                                 xsl(cc, r))
                        t.wait_op(s_pe, n_chain * (r - 2) + cc + 1, "sem-ge")
                    else:
                        t = nc.vector.tensor_mul(esl(cc, r), psl(cc - 2, r),
                                                 xsl(cc, r))
                        t.wait_op(s_ac, n_rt * (r - 2) + (cc - 2) + 1,
                                  "sem-ge")
                    t.then_inc(s_vd, 1)

        @block.gpsimd
        def _(gpsimd):
            # stitch snapshot (segment side) + extraction staging copies
            for k in range(N_ST):
                if RC[k] == 1:
                    # round 1 is E[1] == X'[1], resident in x0buf (fp8)
                    gpsimd.wait_ge(s_x[0], 16)
                    gpsimd.dma_start(snap1[:, :], x0buf[:, 256:256 + NC]
                                     ).then_inc(s_o, 16)
                else:
                    gpsimd.wait_ge(s_vd, n_chain * (RC[k] - 1))
                    gpsimd.dma_start(snaps[:, k * NC:(k + 1) * NC],
                                     Eall[:, RC[k] * NC:(RC[k] + 1) * NC]
                                     ).then_inc(s_o, 16)
            for i, r in enumerate(range(R_EX0, R_EX1 + 1)):
                cp = nc.gpsimd.tensor_copy(stage[:, i * XCOL:(i + 1) * XCOL],
                                           Eall[:, r * NC:r * NC + XCOL])
                cp.wait_op(s_vd, n_chain * (r - 2) + 1, "sem-ge")
                cp.then_inc(s_st, 1)

    return nc


def _simulate_device(xp_core, plan):
    """Numpy emulation of the per-core program (bf16 rounding)."""
    chains = plan["chains"]
    NC = sum(chains)
    bf = ml_dtypes.bfloat16
    xp0_core, xrest_core = xp_core
    wr = xp0_core[:, 0:256].view(np.uint8).view(
        ml_dtypes.bfloat16).astype(np.float32)
    X = np.empty((128, NR, NC), np.float32)
    X[:, 0, :] = np.asarray(xp0_core[:, 256:], np.float32)
    X[:, 1:, :] = np.asarray(xrest_core, np.float32).reshape(
        128, NR - 1, NC)
    Ee = np.ones((L, 128, NC), bf)
    off = np.cumsum([0] + chains)
    Ee[1] = X[:, 0, :]
    for r in range(2, RL + 1):
        psv = wr.T @ np.asarray(Ee[r - 1], np.float32)
        # routed chains rematerialize ps through bf16 before the mul
        psv[:, off[2]:] = psv[:, off[2]:].astype(bf).astype(np.float32)
        Ee[r] = (psv * X[:, r - 1, :]).astype(bf)
    snap1 = xp0_core[:, 256:].copy()     # E[1] == X'[1], exact fp8
    rc1 = Ee[RC[1]] if N_ST > 1 else np.zeros_like(Ee[0])
    rp1 = Ee[RP[1]] if N_ST > 1 else np.zeros_like(Ee[0])
    snaps = np.concatenate([Ee[RC[0]], rc1, Ee[RP[0]], rp1],
                           axis=1).astype(bf)
    stage = np.concatenate(
        [Ee[r][:, 0:XCOL] for r in range(R_EX0, R_EX1 + 1)],
        axis=1).astype(bf)
    return snaps, stage, snap1


def _postprocess(snaps, stage, snap1, plan, core, c):
    """Per-core host math (float64): stitch offsets, read finals."""
    chains = plan["chains"]
    NC = sum(chains)
    off = np.cumsum([0] + chains)
    place = plan["placements"][core]

    lsn = np.log(np.maximum(np.asarray(snaps, np.float64), 1e-300))
    ls1 = np.log(np.maximum(np.asarray(snap1, np.float64), 1e-300))
    lst = np.log(np.maximum(np.asarray(stage, np.float64), 1e-300))

    def cell_snap(cell, k):
        # k: 0,1 -> RC rounds; 2,3 -> RP rounds
        cc, jj, hh = place[cell]
        if k == 0 and RC[0] == 1:
            return ls1[64 * hh:64 * hh + 64, off[cc] + jj]
        return lsn[64 * hh:64 * hh + 64, k * NC + off[cc] + jj]

    A0 = -(W + 1) * c
    res = {}
    for (s_ex, b, r_ex) in plan["extr"][core]:
        A = A0
        for s in range(1, s_ex + 1):
            d = 0.0
            for k in range(N_ST):
                prev = cell_snap((s - 1, b), 2 + k)
                cur = cell_snap((s, b), k)
                d += ((prev + RP[k] * c) - (cur + RC[k] * c)).mean()
            A += d / N_ST
        cc, jj, hh = place[(s_ex, b)]
        assert cc == 0 and jj < XCOL
        lf = lst[64 * hh:64 * hh + 64, (r_ex - R_EX0) * XCOL + jj]
        res[b] = lf.sum() + 64.0 * (r_ex * c + A)
    return res


def kernel(pad_x, transitions, origination, batch_sizes, _simulate=False):
    pad_x = np.asarray(pad_x)
    transitions = np.asarray(transitions)
    origination = np.asarray(origination)
    batch_sizes = np.asarray(batch_sizes)

    c = _c_step(transitions, pad_x)
    plan = _plan(batch_sizes)
    xp, NC = _build_host_inputs(pad_x, transitions, origination, c, plan)

    xp0, xrest = xp
    if _simulate:
        outs = [_simulate_device((xp0[i], xrest[i]), plan)
                for i in range(NCORES)]
    else:
        from concourse.bass_utils import run_bass_kernel_spmd
        key = (batch_sizes.tobytes(), W, SL, N_ST, PH_M)
        if key not in _CACHE:
            _CACHE[key] = _build_program(plan)
        nc = _CACHE[key]
        in_maps = [{"xp0": xp0[i], "xp": xrest[i]} for i in range(NCORES)]
        out = run_bass_kernel_spmd(nc, in_maps, list(range(NCORES)))
        outs = [(out.results[i]["snaps"], out.results[i]["stage"],
                 out.results[i]["snap1"]) for i in range(NCORES)]

    res = np.empty(B, np.float32)
    for i in range(NCORES):
        snaps, stage, snap1 = outs[i]
        for b, v in _postprocess(snaps, stage, snap1, plan, i, c).items():
            res[b] = v
    return res
